# revision 29
# baseline (speedup 1.0000x reference)
"""Dense dilated KNN graph kernel for Trainium2 (8 NeuronCores).

Problem: x (4, 128, 8192, 1) f32, k=16. L2-normalize along channels,
pairwise sq-euclidean distances per batch, top-16 nearest (ascending
distance, ties -> lower index), emit edge_idx (2, B, N, k) int32.

Sharding: 8 cores = 4 batches x 2 query halves. Each core computes a
(4096 x 8192) fused distance + top-16 fully on-chip, never materializing
the distance matrix to HBM:
  score = 2*qhat.T@khat - 2  (monotone per-row transform of -dist, since
  |qhat|^2 = |khat|^2 = 1 up to f32 rounding).
Top-16 is two-level: per-256-key-chunk top-8 (DVE max + max_index per
chunk = 2 row passes), then a 256-candidate merge
(max/max_index/match_replace/max/max_index, 256 wide) and index
recovery via per-slot one-hot select (DVE scalar_tensor_tensor) summed
on the ACT engine. Tie semantics match jax.lax.top_k exactly
(candidate slot order == ascending-index order for equal values).
Ranks 9-16 could only be wrong if one 256-chunk held >=9 of a row's
true top-16 (p ~ 8e-9/row; this problem's fixed input maxes at 7, so
the kernel is exact on it). Residual index flips vs the reference are
fp-rounding near-ties (~3 entries per 1M, the noise floor of any f32
distance recomputation).
"""
import sys

sys.path.insert(0, "/opt/trn_rl_repo")

import numpy as np

B, C, N, K = 4, 128, 8192, 16
NCORES = 8
Q = N // 2          # queries per core
QT = 128            # query tile (partition dim)
NT = Q // QT        # 32 query tiles per core
FCH = 512           # key chunk per matmul (one PSUM bank of f32)
PSW = 2048          # psum tile width (4 banks)
CH = 256            # key chunk for the two-level top-k
NCH = N // CH       # 32 chunks per row
NEG_INF = -3.0e38

_CACHE = {}


def _build():
    if "nc" in _CACHE:
        return _CACHE["nc"]
    import concourse.bass as bass
    import concourse.tile as tile
    from concourse import bacc, mybir

    F32 = mybir.dt.float32
    U32 = mybir.dt.uint32
    ACT = mybir.ActivationFunctionType

    nc = bacc.Bacc(None, target_bir_lowering=False)

    xb_d = nc.declare_dram_parameter("xb", [C, N], F32, isOutput=False)
    xq_d = nc.declare_dram_parameter("xq", [C, Q], F32, isOutput=False)
    cst_d = nc.declare_dram_parameter("cst", [QT, 2 * CH], F32, isOutput=False)
    nn_d = nc.declare_dram_parameter("nn", [QT, NT * K], F32, isOutput=True)

    with tile.TileContext(nc) as tc:
        with (
            tc.tile_pool(name="big", bufs=2) as big,      # X, RSK, then score x2
            tc.tile_pool(name="aux", bufs=1) as aux,
            tc.tile_pool(name="small", bufs=1) as small,
            tc.tile_pool(name="v8p", bufs=4) as v8p,
            tc.tile_pool(name="cand", bufs=1) as cand,
            tc.tile_pool(name="psum", bufs=2, space="PSUM") as psum,
        ):
            X = big.tile([C, N], F32, tag="big")
            RSKb = big.tile([C, N], F32, tag="big")

            XSQ = aux.tile([C, N], F32, tag="a")          # slot reused by Xn
            XQ = aux.tile([C, Q], F32, tag="xq")
            XQSQ = aux.tile([C, Q], F32, tag="c")         # slot reused by XQn2
            RSQb = aux.tile([C, Q], F32, tag="d")

            ones = small.tile([C, 1], F32)
            biasc = small.tile([QT, 1], F32)
            ssk_row = small.tile([1, N], F32, tag="row")
            s128k = small.tile([QT, N // QT], F32, tag="s128k")
            s128q = small.tile([QT, Q // QT], F32, tag="s128q")
            idx = small.tile([QT, NT * K], F32)
            cst = small.tile([QT, 2 * CH], F32)
            msels = []
            for _j in range(4):
                _m = small.tile([QT, NCH * 8], F32, tag=f"msel{_j}",
                                name=f"msel{_j}")
                msels.append(_m)
            asc = small.tile([QT, NCH * 8], F32)

            nc.sync.dma_start(out=cst, in_=cst_d[:])
            for _g in range(8):
                _s = slice(_g * (Q // 8), (_g + 1) * (Q // 8))
                nc.sync.dma_start(out=XQ[:, _s], in_=xq_d[:, _s])
            for _g in range(16):
                _s = slice(_g * (N // 16), (_g + 1) * (N // 16))
                nc.sync.dma_start(out=X[:, _s], in_=xb_d[:, _s])
            nc.vector.memset(ones, 1.0)
            nc.vector.memset(biasc, -2.0)
            wtile = small.tile([C, FCH], F32, tag="warm")
            nc.vector.memset(wtile, 0.0)
            pw = psum.tile([QT, PSW], F32, tag="pp")
            for _w in range(8):
                nc.tensor.matmul(pw[0:1, 0:FCH], ones[:], wtile[:],
                                 start=True, stop=True)

            # ---- key norms: sumsq over channels via ones-matmul ----
            nc.scalar.activation(out=XQSQ, in_=XQ[:], func=ACT.Square)
            for _g in range(8):
                _s = slice(_g * (N // 8), (_g + 1) * (N // 8))
                nc.scalar.activation(out=XSQ[:, _s], in_=X[:, _s], func=ACT.Square)

            for g in range(N // PSW):  # 4 psum rounds for keys
                pp = psum.tile([QT, PSW], F32, tag="pp")
                for j in range(PSW // FCH):
                    f = g * PSW + j * FCH
                    nc.tensor.matmul(
                        pp[0:1, j * FCH:(j + 1) * FCH],
                        ones[:], XSQ[:, f:f + FCH],
                        start=True, stop=True,
                    )
                nc.scalar.activation(
                    out=ssk_row[:, g * PSW:(g + 1) * PSW],
                    in_=pp[0:1, :], func=ACT.Copy,
                )
            nc.sync.dma_start(out=s128k, in_=ssk_row[:])
            ssq_row = small.tile([1, Q], F32, tag="row")
            for g in range(Q // PSW):  # 2 psum rounds for queries
                pp = psum.tile([QT, PSW], F32, tag="pp")
                for j in range(PSW // FCH):
                    f = g * PSW + j * FCH
                    nc.tensor.matmul(
                        pp[0:1, j * FCH:(j + 1) * FCH],
                        ones[:], XQSQ[:, f:f + FCH],
                        start=True, stop=True,
                    )
                nc.scalar.activation(
                    out=ssq_row[:, g * PSW:(g + 1) * PSW],
                    in_=pp[0:1, :], func=ACT.Copy,
                )
            nc.sync.dma_start(out=s128q, in_=ssq_row[:])

            # ---- 1/sqrt on 128 partitions ----
            r128k = small.tile([QT, N // QT], F32, tag="r128k")
            r128q = small.tile([QT, Q // QT], F32, tag="r128q")
            nc.vector.reciprocal(out=r128k, in_=s128k[:])
            nc.vector.reciprocal(out=r128q, in_=s128q[:])
            nc.scalar.activation(out=r128k, in_=r128k[:], func=ACT.Sqrt)
            # 2/norm for queries: sqrt(4*recip) == 2*sqrt(recip) exactly
            nc.scalar.activation(out=r128q, in_=r128q[:], func=ACT.Sqrt,
                                 scale=4.0)

            # ---- broadcast 1/norm to all partitions (chunked SBUF DMAs) ----
            rsk_row = small.tile([1, N], F32, tag="row")
            nc.sync.dma_start(out=rsk_row[:], in_=r128k)
            rsq_row = small.tile([1, Q], F32, tag="row2")
            nc.sync.dma_start(out=rsq_row[:], in_=r128q)
            for g in range(N // FCH):
                _s = slice(g * FCH, (g + 1) * FCH)
                nc.gpsimd.partition_broadcast(RSKb[:, _s], rsk_row[:, _s])
            for g in range(Q // FCH):
                _s = slice(g * FCH, (g + 1) * FCH)
                nc.gpsimd.partition_broadcast(RSQb[:, _s], rsq_row[:, _s])
            Xn = aux.tile([C, N], F32, tag="a")
            for _g in range(4):
                _s = slice(_g * (N // 4), (_g + 1) * (N // 4))
                nc.vector.tensor_mul(Xn[:, _s], X[:, _s], RSKb[:, _s])
            # XQn2 = XQ * (2/norm): the matmul lhsT directly
            XQn2 = aux.tile([C, Q], F32, tag="c")
            nc.vector.tensor_mul(XQn2, XQ[:], RSQb[:])

            # ---- main loop: 32 query tiles ----
            # Two-level top-16: per-256-chunk top-8 (exact unless one chunk
            # holds >=9 of a row's true top-16: p ~ 8e-9 per row; the fixed
            # problem input maxes out at 7), then merge 256 candidates and
            # recover original indices by position masks.
            for t in range(NT):
                score = big.tile([QT, N], F32, tag="big")
                cvals = cand.tile([QT, NCH * 8], F32, tag="cv")
                clidx = cand.tile([QT, NCH * 8], U32, tag="cl")
                cgidx = cand.tile([QT, NCH * 8], F32, tag="cg")
                for g in range(N // PSW):
                    pp = psum.tile([QT, PSW], F32, tag="pp")
                    for j in range(PSW // FCH):
                        f = g * PSW + j * FCH
                        nc.tensor.matmul(
                            pp[:, j * FCH:(j + 1) * FCH],
                            XQn2[:, t * QT:(t + 1) * QT],
                            Xn[:, f:f + FCH],
                            start=True, stop=True,
                        )
                    nc.scalar.activation(
                        out=score[:, g * PSW:(g + 1) * PSW],
                        in_=pp[:], func=ACT.Identity,
                        scale=1.0, bias=biasc[:],
                    )
                    for c in range(g * (PSW // CH), (g + 1) * (PSW // CH)):
                        nc.vector.max(out=cvals[:, 8 * c:8 * c + 8],
                                      in_=score[:, c * CH:(c + 1) * CH])
                        nc.vector.max_index(
                            out=clidx[:, 8 * c:8 * c + 8],
                            in_max=cvals[:, 8 * c:8 * c + 8],
                            in_values=score[:, c * CH:(c + 1) * CH])
                # globalize candidate indices: gidx = f32(lidx) + 256*chunk
                nc.vector.tensor_copy(out=cgidx, in_=clidx[:])
                nc.vector.tensor_add(cgidx, cgidx[:], cst[:, CH:2 * CH])
                # merge 256 candidates -> top-16 values + candidate positions
                v8a = v8p.tile([QT, 8], F32, tag="v8")
                v8b = v8p.tile([QT, 8], F32, tag="v8")
                pos = v8p.tile([QT, K], U32, tag="pos")
                posf = v8p.tile([QT, K], F32, tag="posf")
                nc.vector.max(out=v8a, in_=cvals[:])
                nc.vector.max_index(out=pos[:, 0:8], in_max=v8a[:],
                                    in_values=cvals[:])
                nc.vector.match_replace(out=cvals[:], in_to_replace=v8a[:],
                                        in_values=cvals[:], imm_value=NEG_INF)
                nc.vector.max(out=v8b, in_=cvals[:])
                nc.vector.max_index(out=pos[:, 8:16], in_max=v8b[:],
                                    in_values=cvals[:])
                nc.vector.tensor_copy(out=posf, in_=pos[:])
                # positions -> original indices, slot by slot:
                # m = (iota == pos_s) * gidx  (DVE STT), then ACT sums m
                # into the output slot via accum_out (exactly one nonzero)
                for s in range(K):
                    msel = msels[s % 4]
                    nc.vector.scalar_tensor_tensor(
                        out=msel, in0=cst[:, 0:CH],
                        scalar=posf[:, s:s + 1], in1=cgidx[:],
                        op0=mybir.AluOpType.is_equal,
                        op1=mybir.AluOpType.mult)
                    nc.scalar.activation(
                        out=asc, in_=msel[:], func=ACT.Copy,
                        accum_out=idx[:, t * K + s:t * K + s + 1])

            nc.sync.dma_start(out=nn_d[:], in_=idx)

    nc.compile()
    _CACHE["nc"] = nc
    return nc


def _get_runner():
    """Cached jitted SPMD executor (run_bass_via_pjrt logic, reusable)."""
    if "runner" in _CACHE:
        return _CACHE["runner"]
    import jax
    import numpy as _np
    from jax.sharding import Mesh, PartitionSpec
    from jax.experimental.shard_map import shard_map
    from concourse import bass2jax

    bass2jax.install_neuronx_cc_hook()
    nc = _build()
    in_names = ["xb", "xq", "cst"]
    out_names = ["nn"]
    out_aval = jax.core.ShapedArray((QT, NT * K), _np.float32)

    pid_name = nc.partition_id_tensor.name if nc.partition_id_tensor else None
    bind_names = in_names + out_names + ([pid_name] if pid_name else [])

    def _body(*args):
        operands = list(args)
        if pid_name:
            operands.append(bass2jax.partition_id_tensor())
        outs = bass2jax._bass_exec_p.bind(
            *operands,
            out_avals=(out_aval,),
            in_names=tuple(bind_names),
            out_names=tuple(out_names),
            lowering_input_output_aliases=(),
            sim_require_finite=True,
            sim_require_nnan=True,
            nc=nc,
        )
        return tuple(outs)

    devices = jax.devices()[:NCORES]
    mesh = Mesh(np.asarray(devices), ("core",))
    sharded = jax.jit(
        shard_map(_body, mesh=mesh,
                  in_specs=(PartitionSpec("core"),) * 4,
                  out_specs=(PartitionSpec("core"),),
                  check_rep=False),
        donate_argnums=(3,), keep_unused=True,
    )
    _CACHE["runner"] = sharded
    return sharded


def _cst_block():
    """Per-core constants: [iota 0..255 | chunk offsets 256*(j//8)]."""
    cst = np.empty((QT, 2 * CH), np.float32)
    cst[:, 0:CH] = np.arange(CH, dtype=np.float32)[None, :]
    cst[:, CH:2 * CH] = (CH * (np.arange(CH) // 8)).astype(np.float32)[None, :]
    return cst


def _run(x):
    """x: (B, C, N) f32 contiguous. Returns (8, 128, 512) float32 indices."""
    sharded = _get_runner()
    xb_cat = np.concatenate([x[c // 2] for c in range(NCORES)], axis=0)
    xq_cat = np.concatenate(
        [x[c // 2][:, (c % 2) * Q:(c % 2 + 1) * Q] for c in range(NCORES)],
        axis=0)
    cst_cat = np.concatenate([_cst_block()] * NCORES, axis=0)
    zeros = np.zeros((NCORES * QT, NT * K), np.float32)
    (out,) = sharded(xb_cat, xq_cat, cst_cat, zeros)
    return np.asarray(out).reshape(NCORES, QT, NT * K)


def _run_legacy(x):
    """Fallback: canonical bass_utils.run_bass_kernel_spmd path."""
    from concourse.bass_utils import run_bass_kernel_spmd

    nc = _build()
    in_maps = []
    for c in range(NCORES):
        b, h = divmod(c, 2)
        in_maps.append({
            "xb": np.ascontiguousarray(x[b]),
            "xq": np.ascontiguousarray(x[b][:, h * Q:(h + 1) * Q]),
            "cst": _cst_block(),
        })
    res = run_bass_kernel_spmd(nc, in_maps, list(range(NCORES)))
    return np.stack([res.results[c]["nn"] for c in range(NCORES)])


def kernel(x, k):
    assert x.shape == (B, C, N, 1) and k == K
    x = np.ascontiguousarray(np.asarray(x, dtype=np.float32)[..., 0])  # (B,C,N)
    try:
        outs = _run(x)
    except Exception:
        # transient NRT device errors happen under rapid launches; retry
        # once after a settle, then fall back to the stock SPMD runner
        import time
        time.sleep(3.0)
        try:
            outs = _run(x)
        except Exception:
            time.sleep(3.0)
            outs = _run_legacy(x)

    nn = np.empty((B, N, K), dtype=np.int32)
    for c in range(NCORES):
        b, h = divmod(c, 2)
        out = outs[c].astype(np.int32)  # (128, 512)
        out = out.reshape(QT, NT, K).transpose(1, 0, 2).reshape(Q, K)
        nn[b, h * Q:(h + 1) * Q] = out

    center = np.broadcast_to(
        np.arange(N, dtype=np.int32)[None, :, None], (B, N, K))
    return np.stack((nn, center), axis=0).astype(np.int32)


# revision 31
# speedup vs baseline: 1.0780x; 1.0780x over previous
"""Dense dilated KNN graph kernel for Trainium2 (8 NeuronCores).

Problem: x (4, 128, 8192, 1) f32, k=16. L2-normalize along channels,
pairwise sq-euclidean distances per batch, top-16 nearest (ascending
distance, ties -> lower index), emit edge_idx (2, B, N, k) int32.

Sharding: 8 cores = 4 batches x 2 query halves. Each core computes a
(4096 x 8192) fused distance + top-16 fully on-chip, never materializing
the distance matrix to HBM:
  score = 2*qhat.T@khat - 2  (monotone per-row transform of -dist, since
  |qhat|^2 = |khat|^2 = 1 up to f32 rounding).
Top-16 is two-level: per-256-key-chunk top-8 (DVE max + max_index per
chunk = 2 row passes), then a 256-candidate merge
(max/max_index/match_replace/max/max_index, 256 wide) and index
recovery via per-slot one-hot select (DVE scalar_tensor_tensor) summed
on the ACT engine. Tie semantics match jax.lax.top_k exactly
(candidate slot order == ascending-index order for equal values).
Ranks 9-16 could only be wrong if one 256-chunk held >=9 of a row's
true top-16 (p ~ 8e-9/row; this problem's fixed input maxes at 7, so
the kernel is exact on it). Residual index flips vs the reference are
fp-rounding near-ties (~3 entries per 1M, the noise floor of any f32
distance recomputation).
"""
import sys

sys.path.insert(0, "/opt/trn_rl_repo")

import numpy as np

B, C, N, K = 4, 128, 8192, 16
NCORES = 8
Q = N // 2          # queries per core
QT = 128            # query tile (partition dim)
NT = Q // QT        # 32 query tiles per core
FCH = 512           # key chunk per matmul (one PSUM bank of f32)
PSW = 2048          # psum tile width (4 banks)
CH = 256            # key chunk for the two-level top-k
NCH = N // CH       # 32 chunks per row
NEG_INF = -3.0e38

_CACHE = {}


def _build():
    if "nc" in _CACHE:
        return _CACHE["nc"]
    import concourse.bass as bass
    import concourse.tile as tile
    from concourse import bacc, mybir

    F32 = mybir.dt.float32
    U32 = mybir.dt.uint32
    ACT = mybir.ActivationFunctionType

    nc = bacc.Bacc(None, target_bir_lowering=False)

    xb_d = nc.declare_dram_parameter("xb", [C, N], F32, isOutput=False)
    xq_d = nc.declare_dram_parameter("xq", [C, Q], F32, isOutput=False)
    cst_d = nc.declare_dram_parameter("cst", [QT, 2 * CH], F32, isOutput=False)
    nn_d = nc.declare_dram_parameter("nn", [QT, NT * K], F32, isOutput=True)

    with tile.TileContext(nc) as tc:
        with (
            tc.tile_pool(name="big", bufs=2) as big,      # X, RSK, then score x2
            tc.tile_pool(name="aux", bufs=1) as aux,
            tc.tile_pool(name="small", bufs=1) as small,
            tc.tile_pool(name="v8p", bufs=4) as v8p,
            tc.tile_pool(name="cand", bufs=1) as cand,
            tc.tile_pool(name="psum", bufs=2, space="PSUM") as psum,
        ):
            X = big.tile([C, N], F32, tag="big")
            RSKb = big.tile([C, N], F32, tag="big")

            XSQ = aux.tile([C, N], F32, tag="a")          # slot reused by Xn
            XQ = aux.tile([C, Q], F32, tag="xq")
            XQSQ = aux.tile([C, Q], F32, tag="c")         # slot reused by XQn2
            RSQb = aux.tile([C, Q], F32, tag="d")

            ones = small.tile([C, 1], F32)
            biasc = small.tile([QT, 1], F32)
            ssk_row = small.tile([1, N], F32, tag="row")
            s128k = small.tile([QT, N // QT], F32, tag="s128k")
            s128q = small.tile([QT, Q // QT], F32, tag="s128q")
            idx = small.tile([QT, NT * K], F32)
            cst = small.tile([QT, 2 * CH], F32)
            msels = []
            for _j in range(4):
                _m = small.tile([QT, NCH * 8], F32, tag=f"msel{_j}",
                                name=f"msel{_j}")
                msels.append(_m)
            asc = small.tile([QT, NCH * 8], F32)

            nc.sync.dma_start(out=cst, in_=cst_d[:])
            for _g in range(8):
                _s = slice(_g * (Q // 8), (_g + 1) * (Q // 8))
                nc.sync.dma_start(out=XQ[:, _s], in_=xq_d[:, _s])
            for _g in range(16):
                _s = slice(_g * (N // 16), (_g + 1) * (N // 16))
                nc.sync.dma_start(out=X[:, _s], in_=xb_d[:, _s])
            nc.vector.memset(ones, 1.0)
            nc.vector.memset(biasc, -2.0)
            wtile = small.tile([C, FCH], F32, tag="warm")
            nc.vector.memset(wtile, 0.0)
            pw = psum.tile([QT, PSW], F32, tag="pp")
            for _w in range(8):
                nc.tensor.matmul(pw[0:1, 0:FCH], ones[:], wtile[:],
                                 start=True, stop=True)

            # ---- key norms: sumsq over channels via ones-matmul ----
            nc.scalar.activation(out=XQSQ, in_=XQ[:], func=ACT.Square)
            for _g in range(8):
                _s = slice(_g * (N // 8), (_g + 1) * (N // 8))
                nc.scalar.activation(out=XSQ[:, _s], in_=X[:, _s], func=ACT.Square)

            ssq_row = small.tile([1, Q], F32, tag="row2")
            for g in range(Q // PSW):  # 2 psum rounds for queries
                pp = psum.tile([QT, PSW], F32, tag="pp")
                for j in range(PSW // FCH):
                    f = g * PSW + j * FCH
                    nc.tensor.matmul(
                        pp[0:1, j * FCH:(j + 1) * FCH],
                        ones[:], XQSQ[:, f:f + FCH],
                        start=True, stop=True,
                    )
                nc.scalar.activation(
                    out=ssq_row[:, g * PSW:(g + 1) * PSW],
                    in_=pp[0:1, :], func=ACT.Copy,
                )
            nc.sync.dma_start(out=s128q, in_=ssq_row[:])
            for g in range(N // PSW):  # 4 psum rounds for keys
                pp = psum.tile([QT, PSW], F32, tag="pp")
                for j in range(PSW // FCH):
                    f = g * PSW + j * FCH
                    nc.tensor.matmul(
                        pp[0:1, j * FCH:(j + 1) * FCH],
                        ones[:], XSQ[:, f:f + FCH],
                        start=True, stop=True,
                    )
                nc.scalar.activation(
                    out=ssk_row[:, g * PSW:(g + 1) * PSW],
                    in_=pp[0:1, :], func=ACT.Copy,
                )
            nc.sync.dma_start(out=s128k, in_=ssk_row[:])

            # ---- 1/sqrt on 128 partitions ----
            r128k = small.tile([QT, N // QT], F32, tag="r128k")
            r128q = small.tile([QT, Q // QT], F32, tag="r128q")
            nc.vector.reciprocal(out=r128q, in_=s128q[:])
            nc.vector.reciprocal(out=r128k, in_=s128k[:])
            # 2/norm for queries: sqrt(4*recip) == 2*sqrt(recip) exactly
            nc.scalar.activation(out=r128q, in_=r128q[:], func=ACT.Sqrt,
                                 scale=4.0)
            nc.scalar.activation(out=r128k, in_=r128k[:], func=ACT.Sqrt)

            # ---- broadcast 1/norm to all partitions (chunked SBUF DMAs) ----
            rsq_row = small.tile([1, Q], F32, tag="row2")
            nc.sync.dma_start(out=rsq_row[:], in_=r128q)
            rsk_row = small.tile([1, N], F32, tag="row")
            nc.sync.dma_start(out=rsk_row[:], in_=r128k)
            for g in range(Q // FCH):
                _s = slice(g * FCH, (g + 1) * FCH)
                nc.gpsimd.partition_broadcast(RSQb[:, _s], rsq_row[:, _s])
            # XQn2 = XQ * (2/norm): the matmul lhsT directly
            XQn2 = aux.tile([C, Q], F32, tag="c")
            nc.vector.tensor_mul(XQn2, XQ[:], RSQb[:])
            for g in range(N // FCH):
                _s = slice(g * FCH, (g + 1) * FCH)
                nc.gpsimd.partition_broadcast(RSKb[:, _s], rsk_row[:, _s])
            Xn = aux.tile([C, N], F32, tag="a")
            for _g in range(4):
                _s = slice(_g * (N // 4), (_g + 1) * (N // 4))
                nc.vector.tensor_mul(Xn[:, _s], X[:, _s], RSKb[:, _s])

            # ---- main loop: 32 query tiles ----
            # Two-level top-16: per-256-chunk top-8 (exact unless one chunk
            # holds >=9 of a row's true top-16: p ~ 8e-9 per row; the fixed
            # problem input maxes out at 7), then merge 256 candidates and
            # recover original indices by position masks.
            for t in range(NT):
                score = big.tile([QT, N], F32, tag="big")
                cvals = cand.tile([QT, NCH * 8], F32, tag="cv")
                clidx = cand.tile([QT, NCH * 8], U32, tag="cl")
                cgidx = cand.tile([QT, NCH * 8], F32, tag="cg")
                for g in range(N // PSW):
                    pp = psum.tile([QT, PSW], F32, tag="pp")
                    for j in range(PSW // FCH):
                        f = g * PSW + j * FCH
                        nc.tensor.matmul(
                            pp[:, j * FCH:(j + 1) * FCH],
                            XQn2[:, t * QT:(t + 1) * QT],
                            Xn[:, f:f + FCH],
                            start=True, stop=True,
                        )
                    for j in range(PSW // FCH):
                        f = g * PSW + j * FCH
                        nc.scalar.activation(
                            out=score[:, f:f + FCH],
                            in_=pp[:, j * FCH:(j + 1) * FCH],
                            func=ACT.Identity, scale=1.0, bias=biasc[:],
                        )
                    for c in range(g * (PSW // CH), (g + 1) * (PSW // CH)):
                        nc.vector.max(out=cvals[:, 8 * c:8 * c + 8],
                                      in_=score[:, c * CH:(c + 1) * CH])
                        nc.vector.max_index(
                            out=clidx[:, 8 * c:8 * c + 8],
                            in_max=cvals[:, 8 * c:8 * c + 8],
                            in_values=score[:, c * CH:(c + 1) * CH])
                # globalize candidate indices: gidx = f32(lidx) + 256*chunk
                nc.vector.scalar_tensor_tensor(
                    out=cgidx, in0=clidx[:], scalar=0.0,
                    in1=cst[:, CH:2 * CH],
                    op0=mybir.AluOpType.add, op1=mybir.AluOpType.add)
                # merge 256 candidates -> top-16 values + candidate positions
                v8a = v8p.tile([QT, 8], F32, tag="v8")
                v8b = v8p.tile([QT, 8], F32, tag="v8")
                pos = v8p.tile([QT, K], U32, tag="pos")
                posf = v8p.tile([QT, K], F32, tag="posf")
                nc.vector.max(out=v8a, in_=cvals[:])
                nc.vector.max_index(out=pos[:, 0:8], in_max=v8a[:],
                                    in_values=cvals[:])
                nc.vector.match_replace(out=cvals[:], in_to_replace=v8a[:],
                                        in_values=cvals[:], imm_value=NEG_INF)
                nc.vector.max(out=v8b, in_=cvals[:])
                nc.vector.max_index(out=pos[:, 8:16], in_max=v8b[:],
                                    in_values=cvals[:])
                nc.vector.tensor_copy(out=posf, in_=pos[:])
                # positions -> original indices, slot by slot:
                # m = (iota == pos_s) * gidx  (DVE STT), then ACT sums m
                # into the output slot via accum_out (exactly one nonzero)
                for s in range(K):
                    msel = msels[s % 4]
                    nc.vector.scalar_tensor_tensor(
                        out=msel, in0=cst[:, 0:CH],
                        scalar=posf[:, s:s + 1], in1=cgidx[:],
                        op0=mybir.AluOpType.is_equal,
                        op1=mybir.AluOpType.mult)
                    nc.scalar.activation(
                        out=asc, in_=msel[:], func=ACT.Copy,
                        accum_out=idx[:, t * K + s:t * K + s + 1])

            nc.sync.dma_start(out=nn_d[:], in_=idx)

    nc.compile()
    _CACHE["nc"] = nc
    return nc


def _get_runner():
    """Cached jitted SPMD executor (run_bass_via_pjrt logic, reusable)."""
    if "runner" in _CACHE:
        return _CACHE["runner"]
    import jax
    import numpy as _np
    from jax.sharding import Mesh, PartitionSpec
    from jax.experimental.shard_map import shard_map
    from concourse import bass2jax

    bass2jax.install_neuronx_cc_hook()
    nc = _build()
    in_names = ["xb", "xq", "cst"]
    out_names = ["nn"]
    out_aval = jax.core.ShapedArray((QT, NT * K), _np.float32)

    pid_name = nc.partition_id_tensor.name if nc.partition_id_tensor else None
    bind_names = in_names + out_names + ([pid_name] if pid_name else [])

    def _body(*args):
        operands = list(args)
        if pid_name:
            operands.append(bass2jax.partition_id_tensor())
        outs = bass2jax._bass_exec_p.bind(
            *operands,
            out_avals=(out_aval,),
            in_names=tuple(bind_names),
            out_names=tuple(out_names),
            lowering_input_output_aliases=(),
            sim_require_finite=True,
            sim_require_nnan=True,
            nc=nc,
        )
        return tuple(outs)

    devices = jax.devices()[:NCORES]
    mesh = Mesh(np.asarray(devices), ("core",))
    sharded = jax.jit(
        shard_map(_body, mesh=mesh,
                  in_specs=(PartitionSpec("core"),) * 4,
                  out_specs=(PartitionSpec("core"),),
                  check_rep=False),
        donate_argnums=(3,), keep_unused=True,
    )
    _CACHE["runner"] = sharded
    return sharded


def _cst_block():
    """Per-core constants: [iota 0..255 | chunk offsets 256*(j//8)]."""
    cst = np.empty((QT, 2 * CH), np.float32)
    cst[:, 0:CH] = np.arange(CH, dtype=np.float32)[None, :]
    cst[:, CH:2 * CH] = (CH * (np.arange(CH) // 8)).astype(np.float32)[None, :]
    return cst


def _run(x):
    """x: (B, C, N) f32 contiguous. Returns (8, 128, 512) float32 indices."""
    sharded = _get_runner()
    xb_cat = np.concatenate([x[c // 2] for c in range(NCORES)], axis=0)
    xq_cat = np.concatenate(
        [x[c // 2][:, (c % 2) * Q:(c % 2 + 1) * Q] for c in range(NCORES)],
        axis=0)
    cst_cat = np.concatenate([_cst_block()] * NCORES, axis=0)
    zeros = np.zeros((NCORES * QT, NT * K), np.float32)
    (out,) = sharded(xb_cat, xq_cat, cst_cat, zeros)
    return np.asarray(out).reshape(NCORES, QT, NT * K)


def _run_legacy(x):
    """Fallback: canonical bass_utils.run_bass_kernel_spmd path."""
    from concourse.bass_utils import run_bass_kernel_spmd

    nc = _build()
    in_maps = []
    for c in range(NCORES):
        b, h = divmod(c, 2)
        in_maps.append({
            "xb": np.ascontiguousarray(x[b]),
            "xq": np.ascontiguousarray(x[b][:, h * Q:(h + 1) * Q]),
            "cst": _cst_block(),
        })
    res = run_bass_kernel_spmd(nc, in_maps, list(range(NCORES)))
    return np.stack([res.results[c]["nn"] for c in range(NCORES)])


def kernel(x, k):
    assert x.shape == (B, C, N, 1) and k == K
    x = np.ascontiguousarray(np.asarray(x, dtype=np.float32)[..., 0])  # (B,C,N)
    try:
        outs = _run(x)
    except Exception:
        # transient NRT device errors happen under rapid launches; retry
        # once after a settle, then fall back to the stock SPMD runner
        import time
        time.sleep(3.0)
        try:
            outs = _run(x)
        except Exception:
            time.sleep(3.0)
            outs = _run_legacy(x)

    nn = np.empty((B, N, K), dtype=np.int32)
    for c in range(NCORES):
        b, h = divmod(c, 2)
        out = outs[c].astype(np.int32)  # (128, 512)
        out = out.reshape(QT, NT, K).transpose(1, 0, 2).reshape(Q, K)
        nn[b, h * Q:(h + 1) * Q] = out

    center = np.broadcast_to(
        np.arange(N, dtype=np.int32)[None, :, None], (B, N, K))
    return np.stack((nn, center), axis=0).astype(np.int32)
